# revision 6
# baseline (speedup 1.0000x reference)
"""Causal self-attention kernel for 8 TRN2 NeuronCores.

Sharding: data-parallel over batch (B=8 -> 1 batch element per core).
Each core computes full 16-head causal attention for its batch element.
All matmuls run in bf16 with fp32 PSUM accumulation (~5.5e-3 rel err).

Per-core dataflow (L=1024, E=1024, H=16, D=64):
  XT  = x^T           host-pre-transposed bf16, loaded in (l,ct) chunks
  V   = (x Wv + bv)|1   [l, h, 65] layout; the ones column makes the
                        attention matmul emit softmax denominators for free
  QT  = Wq^T x^T + bq   [e, l] layout (stationary Wq blocks, moving XT)
  KT  = Wk^T x^T + bk   [e, l] layout
  S^T = K Q^T           per (head, k-tile): [k=128, q<=1024] PSUM tiles,
                        contraction d=64; only causal tiles computed
  P^T = exp(S^T*scale)  ScalarE exp -> bf16 (scores bounded, no max sub);
                        diagonal tile masked by a 0/1 multiply on GpSimd
                        (keeps the exp->mask->AV chain off the busy VectorE)
  Yu  = [V|1]^T P^T     accumulated over k-tiles; row 64 = softmax sum s
  Y   = Yu[0:64] / s    s DMA'd straight from its PSUM row to DRAM, SWDGE
                        broadcast back to 64 partitions, approx reciprocal
  out = Y^T.T Wo + bo   accumulation over e-tiles

Scheduling (the point of this version): the Tensor queue is in-order, so
emission order is the schedule.  Per (head, k-tile) slot the emission is
  S(kt+2-ahead) | 2 interleaved QK-projection matmuls for e-tile et+1 |
  AV(kt)
which gives ScalarE's exp (0.9 ns/col + 252 ns/instr, slower than the
S+AV matmuls it feeds) a ~1.7 us TensorE shadow per tile, so the AV
matmul's semaphore wait is satisfied before TensorE reaches it.  The
last pair (et=7) is padded with two pre-opened out-projection
accumulation groups (their et=0..6 contributions).  Input DMAs are
chunked (wv/xt per contraction tile) so the first V-proj matmul starts
as soon as ~200 KB has landed.

Measured baseline (sequential heads, coarse proj blocks): ~240-242 us.
"""

import os
import sys

sys.path.insert(0, "/opt/trn_rl_repo")

from collections import deque
from contextlib import ExitStack

import numpy as np

import concourse.bass as bass
import concourse.mybir as mybir
import concourse.tile as tile
from concourse import bacc
from concourse.bass_utils import run_bass_kernel_spmd

f32 = mybir.dt.float32
bf16 = mybir.dt.bfloat16
AF = mybir.ActivationFunctionType
OP = mybir.AluOpType

L = 1024
E = 1024
H = 16
D = 64
P = 128
NT = L // P  # 8 tiles along any 1024 dim
SCALE = 1.0 / np.sqrt(D)


def _build():
    nc = bacc.Bacc("TRN2", target_bir_lowering=False, debug=False, num_devices=8)
    x = nc.dram_tensor("x", [L, E], bf16, kind="ExternalInput").ap()
    wq = nc.dram_tensor("wq", [NT, P, NT, P], bf16, kind="ExternalInput").ap()
    wk = nc.dram_tensor("wk", [NT, P, NT, P], bf16, kind="ExternalInput").ap()
    wv = nc.dram_tensor("wv", [P, NT, E], bf16, kind="ExternalInput").ap()
    wo = nc.dram_tensor("wo", [P, NT, E], bf16, kind="ExternalInput").ap()
    bq = nc.dram_tensor("bq", [E], f32, kind="ExternalInput").ap()
    bk = nc.dram_tensor("bk", [E], f32, kind="ExternalInput").ap()
    bv = nc.dram_tensor("bv", [E], f32, kind="ExternalInput").ap()
    bo = nc.dram_tensor("bo", [E], f32, kind="ExternalInput").ap()
    xt_d = nc.dram_tensor("xt", [P, NT, L], bf16, kind="ExternalInput").ap()
    mask_d = nc.dram_tensor("mask01", [P, P], bf16, kind="ExternalInput").ap()
    out = nc.dram_tensor("out", [L, E], f32, kind="ExternalOutput").ap()
    s_dram = nc.dram_tensor("s_scratch", [H, L], f32, kind="Internal").ap()

    with tile.TileContext(nc) as tc:
        _body(nc, tc, wq, wk, wv, wo, bq, bk, bv, bo, out, s_dram,
              xt_d, mask_d)
    return nc


def _body(nc, tc, wq, wk, wv, wo, bq, bk, bv, bo, out, s_dram, xt_d, mask_d):
    ctx = ExitStack()
    with ctx:
        consts = ctx.enter_context(tc.tile_pool(name="consts", bufs=1))
        qt_pool = ctx.enter_context(tc.tile_pool(name="qt_pool", bufs=1))
        kt_pool = ctx.enter_context(tc.tile_pool(name="kt_pool", bufs=1))
        v_pool = ctx.enter_context(tc.tile_pool(name="v_pool", bufs=1))
        y_pool = ctx.enter_context(tc.tile_pool(name="y_pool", bufs=1))
        xt_pool = ctx.enter_context(tc.tile_pool(name="xt_pool", bufs=1))
        wv_pool = ctx.enter_context(tc.tile_pool(name="wv_pool", bufs=1))
        r_pool = ctx.enter_context(tc.tile_pool(name="r_pool", bufs=1))
        wo_pool = ctx.enter_context(tc.tile_pool(name="wo_pool", bufs=1))
        wblk_pool = ctx.enter_context(tc.tile_pool(name="wblk_pool", bufs=4))
        pt_pool = ctx.enter_context(tc.tile_pool(name="pt_pool", bufs=4))
        osb_pool = ctx.enter_context(tc.tile_pool(name="osb_pool", bufs=3))
        sr_pool = ctx.enter_context(tc.tile_pool(name="sr_pool", bufs=4))
        # PSUM: st 2x[128,1024]=4 banks, yu 2x[65,512]=2, pp 2x[128,512]=2
        pp = ctx.enter_context(tc.tile_pool(name="pp", bufs=2, space="PSUM"))
        sp = ctx.enter_context(tc.tile_pool(name="sp", bufs=2, space="PSUM"))
        yp = ctx.enter_context(tc.tile_pool(name="yp", bufs=2, space="PSUM"))

        mask01 = consts.tile([P, P], bf16)
        nc.sync.dma_start(out=mask01, in_=mask_d)
        bq_sb = consts.tile([P, NT], f32)
        nc.sync.dma_start(out=bq_sb, in_=bq.rearrange("(et p) -> p et", p=P))
        bk_sb = consts.tile([P, NT], f32)
        nc.sync.dma_start(out=bk_sb, in_=bk.rearrange("(et p) -> p et", p=P))

        QT = qt_pool.tile([P, NT, L], bf16)  # [p, et, l] = Q^T[et*128+p, l]
        KT = kt_pool.tile([P, NT, L], bf16)
        V = v_pool.tile([P, NT, H, D + 1], bf16)  # [p(l), lt, h, d | ones]
        Y = y_pool.tile([P, NT, L], bf16)  # [p, et, l] = y^T[et*128+p, l]
        R = r_pool.tile([P, NT, L], f32)
        XT = xt_pool.tile([P, NT, L], bf16)  # [p, ct, l] = x^T[ct*128+p, l]
        wv_sb = wv_pool.tile([P, NT, E], bf16)
        wo_r = wo_pool.tile([P, NT, E], bf16)

        nc.vector.memset(V[:, :, :, D : D + 1], 1.0)

        # ---- input DMAs, chunked so the first matmul starts early ----
        # pairs (wv ct-chunk of the ec=0 half, XT ct-chunk of l=0:256)
        for ct in range(NT):
            nc.sync.dma_start(out=wv_sb[:, ct, 0:512], in_=wv[:, ct, 0:512])
            nc.sync.dma_start(out=XT[:, ct, 0:256], in_=xt_d[:, ct, 0:256])
        bv_bc = consts.tile([P, E], f32)
        nc.gpsimd.dma_start(
            out=bv_bc,
            in_=bass.AP(tensor=bv.tensor, offset=bv.offset, ap=[[0, P], [1, E]]),
        )
        # rest of XT in l-major chunks (V-proj consumes l-tiles in order)
        for ls in range(1, 4):
            for ct in range(NT):
                nc.sync.dma_start(
                    out=XT[:, ct, ls * 256 : (ls + 1) * 256],
                    in_=xt_d[:, ct, ls * 256 : (ls + 1) * 256],
                )
        nc.sync.dma_start(out=wv_sb[:, :, 512:1024], in_=wv[:, :, 512:1024])

        # ---- V = x @ Wv + bv ----
        for ec in range(2):
            for lt in range(NT):
                ps = pp.tile([P, 512], f32, tag="pp")
                for ct in range(NT):
                    nc.tensor.matmul(
                        ps,
                        XT[:, ct, lt * P : (lt + 1) * P],
                        wv_sb[:, ct, ec * 512 : (ec + 1) * 512],
                        start=(ct == 0),
                        stop=(ct == NT - 1),
                    )
                nc.vector.tensor_tensor(
                    out=V[:, lt, ec * 8 : (ec + 1) * 8, 0:D],
                    in0=ps.rearrange("p (h d) -> p h d", h=8),
                    in1=bv_bc[:, ec * 512 : (ec + 1) * 512].rearrange(
                        "p (h d) -> p h d", h=8
                    ),
                    op=OP.add,
                )

        def qk_proj_thunks(et):
            """Emit weight DMAs for e-tile et now; return 32 one-matmul
            thunks computing QT/KT[:, et, :] when called in order."""
            thunks = []
            for (w_dram, b_sb, dst) in ((wq, bq_sb, QT), (wk, bk_sb, KT)):
                blk = wblk_pool.tile([P, NT, P], bf16, tag="wqkblk")
                nc.sync.dma_start(out=blk, in_=w_dram[et])
                for lc in range(2):
                    grp = {}

                    def t(ct, lc=lc, blk=blk, grp=grp, b_sb=b_sb, dst=dst,
                          et=et):
                        if ct == 0:
                            grp["ps"] = pp.tile([P, 512], f32, tag="pp", name="ps_qk")
                        nc.tensor.matmul(
                            grp["ps"],
                            blk[:, ct, :],
                            XT[:, ct, lc * 512 : (lc + 1) * 512],
                            start=(ct == 0),
                            stop=(ct == NT - 1),
                        )
                        if ct == NT - 1:
                            nc.vector.tensor_scalar(
                                out=dst[:, et, lc * 512 : (lc + 1) * 512],
                                in0=grp["ps"],
                                scalar1=b_sb[:, et : et + 1],
                                scalar2=None,
                                op0=OP.add,
                            )

                    for ct in range(NT):
                        thunks.append(lambda ct=ct, t=t: t(ct))
            return thunks

        # ---- QT/KT for et=0 (unpadded; runs while attention warms up) ----
        for t in qk_proj_thunks(0):
            t()

        # out-projection accumulation state for the et=7 padding trick
        oproj = {}

        def outproj_open_thunks():
            """Thunks accumulating et=0..6 contributions of the first two
            out-proj groups; they pad the last attention pair."""
            thunks = []
            for (lt, oc) in ((0, 0), (0, 1)):
                grp = {}
                oproj[(lt, oc)] = grp

                def t(et, lt=lt, oc=oc, grp=grp):
                    if et == 0:
                        grp["ps"] = pp.tile(
                            [P, 512], f32, tag="pp", name="ps_op")
                    nc.tensor.matmul(
                        grp["ps"],
                        Y[:, et, lt * P : (lt + 1) * P],
                        wo_r[:, et, oc * 512 : (oc + 1) * 512],
                        start=(et == 0),
                        stop=False,
                    )

                for et in range(NT - 1):
                    thunks.append(lambda et=et, t=t: t(et))
            return thunks

        def head_tail(h, qc, yu_t):
            et = h // 2
            pb = (h % 2) * 64
            cols = slice(qc * 512, (qc + 1) * 512)
            # softmax denominators: PSUM row -> SBUF -> DRAM, for the
            # SWDGE partition-broadcast (GpSimd cannot read PSUM)
            srow = sr_pool.tile([1, 512], f32, tag="srow", name="srow")
            nc.vector.tensor_copy(out=srow, in_=yu_t[D : D + 1, :])
            nc.sync.dma_start(out=s_dram[h : h + 1, cols], in_=srow)
            # unnormalized y -> bf16 SBUF (normalized in place later)
            nc.vector.tensor_copy(out=Y[pb : pb + D, et, cols], in_=yu_t[0:D, :])

        def attention_head(h, pads, n_pads):
            et = h // 2
            pb = (h % 2) * 64
            yu = [None, None]
            pts = {}

            def emit_S(kt):
                qlen = L - kt * P
                st = sp.tile([P, L], f32, tag="st", name="st")
                for s0 in range(0, qlen, 512):
                    n = min(512, qlen - s0)
                    nc.tensor.matmul(
                        st[:, s0 : s0 + n],
                        KT[pb : pb + D, et, kt * P : (kt + 1) * P],
                        QT[pb : pb + D, et, kt * P + s0 : kt * P + s0 + n],
                        start=True,
                        stop=True,
                    )
                pt = pt_pool.tile([P, L], bf16, tag="pt", name="pt")
                nc.scalar.activation(
                    out=pt[:, 0:qlen], in_=st[:, 0:qlen], func=AF.Exp,
                    scale=float(SCALE),
                )
                # causal mask on the diagonal tile (GpSimd: off VectorE's
                # queue so the exp->mask->AV chain never queues)
                nc.gpsimd.tensor_tensor(
                    out=pt[:, 0:P], in0=pt[:, 0:P], in1=mask01, op=OP.mult
                )
                pts[kt] = pt

            def emit_A(kt):
                for qc in range(2):
                    lo = max(qc * 512, kt * P)
                    hi = (qc + 1) * 512
                    if lo >= hi:
                        continue
                    if kt == 0 and yu[qc] is None:
                        yu[qc] = yp.tile([D + 1, 512], f32, tag="yu", name="yu")
                    last_kt = min(NT - 1, (qc + 1) * 4 - 1)
                    nc.tensor.matmul(
                        yu[qc][:, lo - qc * 512 : hi - qc * 512],
                        V[:, kt, h, :],
                        pts[kt][:, lo - kt * P : hi - kt * P],
                        start=(kt == 0),
                        stop=(kt == last_kt),
                    )
                    if kt == last_kt:
                        head_tail(h, qc, yu[qc])

            emit_S(0)
            emit_S(1)
            for kt in range(NT):
                for _ in range(n_pads):
                    if pads:
                        pads.popleft()()
                emit_A(kt)
                if kt + 2 < NT:
                    emit_S(kt + 2)

        def normalize_et(et):
            for half in range(2):
                hh = 2 * et + half
                bsrc = bass.AP(
                    tensor=s_dram.tensor,
                    offset=s_dram[hh : hh + 1, :].offset,
                    ap=[[0, 64], [1, L]],
                )
                nc.gpsimd.dma_start(
                    out=R[half * 64 : (half + 1) * 64, et, :], in_=bsrc
                )
            nc.vector.reciprocal_approx_fast(out=R[:, et, :], in_=R[:, et, :])
            for half in range(2):
                rows = slice(half * 64, (half + 1) * 64)
                nc.vector.tensor_tensor(
                    out=Y[rows, et, :],
                    in0=Y[rows, et, :],
                    in1=R[rows, et, :],
                    op=OP.mult,
                )

        # ---- et loop: attention pair + interleaved next-et projections ----
        for et in range(NT):
            if et == 2:
                # weight prefetches for the output projection, mid-loop
                nc.sync.dma_start(out=wo_r, in_=wo)
                bo_bc = consts.tile([P, E], f32)
                nc.gpsimd.dma_start(
                    out=bo_bc,
                    in_=bass.AP(
                        tensor=bo.tensor, offset=bo.offset, ap=[[0, P], [1, E]]
                    ),
                )
            if et + 1 < NT:
                pads = deque(qk_proj_thunks(et + 1))
                n_pads = 2
            else:
                pads = deque(outproj_open_thunks())
                n_pads = 1
            attention_head(2 * et, pads, n_pads)
            attention_head(2 * et + 1, pads, n_pads)
            while pads:
                pads.popleft()()
            normalize_et(et)

        # ---- out = Y^T.T @ Wo + bo ----
        def close_group(lt, oc, ps):
            nc.tensor.matmul(
                ps,
                Y[:, NT - 1, lt * P : (lt + 1) * P],
                wo_r[:, NT - 1, oc * 512 : (oc + 1) * 512],
                start=False,
                stop=True,
            )
            osb = osb_pool.tile([P, 512], f32)
            nc.vector.tensor_tensor(
                out=osb, in0=ps, in1=bo_bc[:, oc * 512 : (oc + 1) * 512],
                op=OP.add,
            )
            nc.sync.dma_start(
                out=out[lt * P : (lt + 1) * P, oc * 512 : (oc + 1) * 512],
                in_=osb,
            )

        for (lt, oc) in ((0, 0), (0, 1)):
            close_group(lt, oc, oproj[(lt, oc)]["ps"])
        for lt in range(NT):
            for oc in range(2):
                if (lt, oc) in oproj:
                    continue
                ps = pp.tile([P, 512], f32, tag="pp")
                for et in range(NT):
                    nc.tensor.matmul(
                        ps,
                        Y[:, et, lt * P : (lt + 1) * P],
                        wo_r[:, et, oc * 512 : (oc + 1) * 512],
                        start=(et == 0),
                        stop=(et == NT - 1),
                    )
                osb = osb_pool.tile([P, 512], f32)
                nc.vector.tensor_tensor(
                    out=osb, in0=ps, in1=bo_bc[:, oc * 512 : (oc + 1) * 512],
                    op=OP.add,
                )
                nc.sync.dma_start(
                    out=out[lt * P : (lt + 1) * P, oc * 512 : (oc + 1) * 512],
                    in_=osb,
                )


_COMPILED = None


def _get_compiled():
    global _COMPILED
    if _COMPILED is None:
        nc = _build()
        nc.compile()
        _COMPILED = nc
    return _COMPILED


def kernel(x, Wq, bq, Wk, bk, Wv, bv, Wo, bo, _trace=False):
    import ml_dtypes

    bfl = ml_dtypes.bfloat16
    nc = _get_compiled()
    x = np.ascontiguousarray(np.asarray(x, dtype=np.float32).astype(bfl))
    B = x.shape[0]
    assert B == 8 and x.shape[1] == L and x.shape[2] == E

    def _qk_layout(w):
        # [et, p, ct, e']: per-et contiguous [128, 8, 128] stationary blocks
        w = np.asarray(w, np.float32).astype(bfl)
        return np.ascontiguousarray(
            w.reshape(NT, P, NT, P).transpose(2, 1, 0, 3)
        )

    def _pct_layout(w):
        # [p, ct, e]: moving-operand blocks with contraction rows on partitions
        w = np.asarray(w, np.float32).astype(bfl)
        return np.ascontiguousarray(w.reshape(NT, P, E).transpose(1, 0, 2))

    common = {
        "wq": _qk_layout(Wq),
        "wk": _qk_layout(Wk),
        "wv": _pct_layout(Wv),
        "wo": _pct_layout(Wo),
        "bq": np.ascontiguousarray(np.asarray(bq, np.float32)),
        "bk": np.ascontiguousarray(np.asarray(bk, np.float32)),
        "bv": np.ascontiguousarray(np.asarray(bv, np.float32)),
        "bo": np.ascontiguousarray(np.asarray(bo, np.float32)),
    }
    common["mask01"] = np.tril(np.ones((P, P), np.float32)).T.astype(bfl)
    # xt[b]: [p, ct, l] with xt[b][p, ct, l] = x[b, l, ct*128+p]
    xt = np.ascontiguousarray(
        x.transpose(0, 2, 1).reshape(B, NT, P, L).transpose(0, 2, 1, 3)
    )
    in_maps = [dict(common, x=x[i], xt=xt[i]) for i in range(B)]
    res = run_bass_kernel_spmd(nc, in_maps, core_ids=list(range(8)), trace=_trace)
    outp = np.stack([res.results[i]["out"] for i in range(B)])
    if _trace:
        kernel.last_exec_time_ns = res.exec_time_ns
    return outp


# revision 7
# speedup vs baseline: 1.1348x; 1.1348x over previous
"""Causal self-attention kernel for 8 TRN2 NeuronCores.

Sharding: data-parallel over batch (B=8 -> 1 batch element per core).
Each core computes full 16-head causal attention for its batch element.
All matmuls run in bf16 with fp32 PSUM accumulation (~5.5e-3 rel err).

Per-core dataflow (L=1024, E=1024, H=16, D=64):
  XT  = x^T           host-pre-transposed bf16, loaded in (l,ct) chunks
  V   = (x Wv + bv)|1   [l, h, 65] layout; the ones column makes the
                        attention matmul emit softmax denominators for free
  QT  = Wq^T x^T + bq   [e, l] layout (stationary Wq blocks, moving XT)
  KT  = Wk^T x^T + bk   [e, l] layout
  S^T = K Q^T           per (head, k-tile): [k=128, q<=1024] PSUM tiles,
                        contraction d=64; only causal tiles computed
  P^T = exp(S^T*scale)  ScalarE exp -> bf16 (scores bounded, no max sub);
                        diagonal tile masked by a 0/1 multiply on GpSimd
                        (keeps the exp->mask->AV chain off the busy VectorE)
  Yu  = [V|1]^T P^T     accumulated over k-tiles; row 64 = softmax sum s
  Y   = Yu[0:64] / s    s DMA'd straight from its PSUM row to DRAM, SWDGE
                        broadcast back to 64 partitions, approx reciprocal
  out = Y^T.T Wo + bo   accumulation over e-tiles

Scheduling (the point of this version): the Tensor queue is in-order, so
emission order is the schedule.  Per (head, k-tile) slot the emission is
  S(kt+2-ahead) | 2 interleaved QK-projection matmuls for e-tile et+1 |
  AV(kt)
which gives ScalarE's exp (0.9 ns/col + 252 ns/instr, slower than the
S+AV matmuls it feeds) a ~1.7 us TensorE shadow per tile, so the AV
matmul's semaphore wait is satisfied before TensorE reaches it.  The
last pair (et=7) is padded with two pre-opened out-projection
accumulation groups (their et=0..6 contributions).  Input DMAs are
chunked (wv/xt per contraction tile) so the first V-proj matmul starts
as soon as ~200 KB has landed.

Measured baseline (sequential heads, coarse proj blocks): ~240-242 us.
"""

import os
import sys

sys.path.insert(0, "/opt/trn_rl_repo")

from collections import deque
from contextlib import ExitStack

import numpy as np

import concourse.bass as bass
import concourse.mybir as mybir
import concourse.tile as tile
from concourse import bacc
from concourse.bass_utils import run_bass_kernel_spmd

f32 = mybir.dt.float32
bf16 = mybir.dt.bfloat16
AF = mybir.ActivationFunctionType
OP = mybir.AluOpType

L = 1024
E = 1024
H = 16
D = 64
P = 128
NT = L // P  # 8 tiles along any 1024 dim
SCALE = 1.0 / np.sqrt(D)


def _build():
    nc = bacc.Bacc("TRN2", target_bir_lowering=False, debug=False, num_devices=8)
    x = nc.dram_tensor("x", [L, E], bf16, kind="ExternalInput").ap()
    wq = nc.dram_tensor("wq", [NT, P, NT, P], bf16, kind="ExternalInput").ap()
    wk = nc.dram_tensor("wk", [NT, P, NT, P], bf16, kind="ExternalInput").ap()
    wv = nc.dram_tensor("wv", [P, NT, E], bf16, kind="ExternalInput").ap()
    wo = nc.dram_tensor("wo", [P, NT, E], bf16, kind="ExternalInput").ap()
    bq = nc.dram_tensor("bq", [E], f32, kind="ExternalInput").ap()
    bk = nc.dram_tensor("bk", [E], f32, kind="ExternalInput").ap()
    bv = nc.dram_tensor("bv", [E], f32, kind="ExternalInput").ap()
    bo = nc.dram_tensor("bo", [E], f32, kind="ExternalInput").ap()
    xt_d = nc.dram_tensor("xt", [P, NT, L], bf16, kind="ExternalInput").ap()
    mask_d = nc.dram_tensor("mask01", [P, P], bf16, kind="ExternalInput").ap()
    out = nc.dram_tensor("out", [L, E], f32, kind="ExternalOutput").ap()
    s_dram = nc.dram_tensor("s_scratch", [H, L], f32, kind="Internal").ap()

    with tile.TileContext(nc) as tc:
        _body(nc, tc, wq, wk, wv, wo, bq, bk, bv, bo, out, s_dram,
              xt_d, mask_d)
    return nc


def _body(nc, tc, wq, wk, wv, wo, bq, bk, bv, bo, out, s_dram, xt_d, mask_d):
    ctx = ExitStack()
    with ctx:
        consts = ctx.enter_context(tc.tile_pool(name="consts", bufs=1))
        qt_pool = ctx.enter_context(tc.tile_pool(name="qt_pool", bufs=1))
        kt_pool = ctx.enter_context(tc.tile_pool(name="kt_pool", bufs=1))
        v_pool = ctx.enter_context(tc.tile_pool(name="v_pool", bufs=1))
        y_pool = ctx.enter_context(tc.tile_pool(name="y_pool", bufs=1))
        xt_pool = ctx.enter_context(tc.tile_pool(name="xt_pool", bufs=1))
        wv_pool = ctx.enter_context(tc.tile_pool(name="wv_pool", bufs=1))
        r_pool = ctx.enter_context(tc.tile_pool(name="r_pool", bufs=1))
        wo_pool = ctx.enter_context(tc.tile_pool(name="wo_pool", bufs=1))
        wblk_pool = ctx.enter_context(tc.tile_pool(name="wblk_pool", bufs=4))
        pt_pool = ctx.enter_context(tc.tile_pool(name="pt_pool", bufs=4))
        osb_pool = ctx.enter_context(tc.tile_pool(name="osb_pool", bufs=3))
        sr_pool = ctx.enter_context(tc.tile_pool(name="sr_pool", bufs=4))
        # PSUM: st 2x[128,1024]=4 banks, yu 2x[65,512]=2, pp 2x[128,512]=2
        pp = ctx.enter_context(tc.tile_pool(name="pp", bufs=2, space="PSUM"))
        sp = ctx.enter_context(tc.tile_pool(name="sp", bufs=2, space="PSUM"))
        yp = ctx.enter_context(tc.tile_pool(name="yp", bufs=2, space="PSUM"))

        mask01 = consts.tile([P, P], bf16)
        nc.sync.dma_start(out=mask01, in_=mask_d)
        bq_sb = consts.tile([P, NT], f32)
        nc.sync.dma_start(out=bq_sb, in_=bq.rearrange("(et p) -> p et", p=P))
        bk_sb = consts.tile([P, NT], f32)
        nc.sync.dma_start(out=bk_sb, in_=bk.rearrange("(et p) -> p et", p=P))
        ones_t = consts.tile([D + 1, P], bf16)
        nc.vector.memset(ones_t, 0.0)
        nc.vector.memset(ones_t[D : D + 1, :], 1.0)

        QT = qt_pool.tile([P, NT, L], bf16)  # [p, et, l] = Q^T[et*128+p, l]
        KT = kt_pool.tile([P, NT, L], bf16)
        V = v_pool.tile([P, NT, H, D + 1], bf16)  # [p(l), lt, h, d | ones]
        Y = y_pool.tile([P, NT, L], bf16)  # [p, et, l] = y^T[et*128+p, l]
        R = r_pool.tile([P, NT, L], f32)
        XT = xt_pool.tile([P, NT, L], bf16)  # [p, ct, l] = x^T[ct*128+p, l]
        wv_sb = wv_pool.tile([P, NT, E], bf16)
        wo_r = wo_pool.tile([P, NT, E], bf16)

        nc.vector.memset(V[:, :, :, D : D + 1], 1.0)

        # ---- input DMAs, ordered by first use (big transfers keep the
        # descriptor stream efficient; per-ct chunking measured ~2x slower)
        nc.sync.dma_start(out=XT[:, :, 0:256], in_=xt_d[:, :, 0:256])
        nc.sync.dma_start(out=wv_sb[:, :, 0:512], in_=wv[:, :, 0:512])
        bv_bc = consts.tile([P, E], f32)
        nc.gpsimd.dma_start(
            out=bv_bc,
            in_=bass.AP(tensor=bv.tensor, offset=bv.offset, ap=[[0, P], [1, E]]),
        )
        nc.sync.dma_start(out=XT[:, :, 256:512], in_=xt_d[:, :, 256:512])
        nc.sync.dma_start(out=XT[:, :, 512:1024], in_=xt_d[:, :, 512:1024])
        nc.sync.dma_start(out=wv_sb[:, :, 512:1024], in_=wv[:, :, 512:1024])

        # ---- V = x @ Wv + bv ----
        for ec in range(2):
            for lt in range(NT):
                ps = pp.tile([P, 512], f32, tag="pp")
                for ct in range(NT):
                    nc.tensor.matmul(
                        ps,
                        XT[:, ct, lt * P : (lt + 1) * P],
                        wv_sb[:, ct, ec * 512 : (ec + 1) * 512],
                        start=(ct == 0),
                        stop=(ct == NT - 1),
                    )
                nc.vector.tensor_tensor(
                    out=V[:, lt, ec * 8 : (ec + 1) * 8, 0:D],
                    in0=ps.rearrange("p (h d) -> p h d", h=8),
                    in1=bv_bc[:, ec * 512 : (ec + 1) * 512].rearrange(
                        "p (h d) -> p h d", h=8
                    ),
                    op=OP.add,
                )

        def qk_proj_thunks(et):
            """Emit weight DMAs for e-tile et now; return 32 one-matmul
            thunks computing QT/KT[:, et, :] when called in order."""
            thunks = []
            for (w_dram, b_sb, dst) in ((wq, bq_sb, QT), (wk, bk_sb, KT)):
                blk = wblk_pool.tile([P, NT, P], bf16, tag="wqkblk")
                nc.sync.dma_start(out=blk, in_=w_dram[et])
                for lc in range(2):
                    grp = {}

                    def t(ct, lc=lc, blk=blk, grp=grp, b_sb=b_sb, dst=dst,
                          et=et):
                        if ct == 0:
                            grp["ps"] = pp.tile([P, 512], f32, tag="pp", name="ps_qk")
                        nc.tensor.matmul(
                            grp["ps"],
                            blk[:, ct, :],
                            XT[:, ct, lc * 512 : (lc + 1) * 512],
                            start=(ct == 0),
                            stop=(ct == NT - 1),
                        )
                        if ct == NT - 1:
                            nc.vector.tensor_scalar(
                                out=dst[:, et, lc * 512 : (lc + 1) * 512],
                                in0=grp["ps"],
                                scalar1=b_sb[:, et : et + 1],
                                scalar2=None,
                                op0=OP.add,
                            )

                    for ct in range(NT):
                        thunks.append(lambda ct=ct, t=t: t(ct))
            return thunks

        # ---- QT/KT for et=0 (unpadded; runs while attention warms up) ----
        for t in qk_proj_thunks(0):
            t()

        def head_tail(h, qc, yu_t, last_pair):
            et = h // 2
            pb = (h % 2) * 64
            cols = slice(qc * 512, (qc + 1) * 512)
            if last_pair:
                # PE-broadcast normalize: avoids the DRAM round-trip latency
                # right before the out-projection consumes Y[:, 7, :]
                sstb = sr_pool.tile([D + 1, 512], bf16, tag="sstb", name="sstb")
                nc.vector.tensor_copy(
                    out=sstb[D : D + 1, :], in_=yu_t[D : D + 1, :]
                )
                ps_bc = pp.tile([P, 512], f32, tag="pp", name="ps_bc")
                nc.tensor.matmul(
                    ps_bc, ones_t[D : D + 1, :], sstb[D : D + 1, :],
                    start=True, stop=True,
                )
                rh = sr_pool.tile([P, 512], f32, tag="rh", name="rh")
                nc.vector.reciprocal_approx_fast(out=rh, in_=ps_bc)
                nc.vector.tensor_tensor(
                    out=Y[pb : pb + D, et, cols],
                    in0=yu_t[0:D, :],
                    in1=rh[0:D, :],
                    op=OP.mult,
                )
                return
            # softmax denominators: PSUM row -> SBUF -> DRAM, for the
            # SWDGE partition-broadcast (GpSimd cannot read PSUM)
            srow = sr_pool.tile([1, 512], f32, tag="srow", name="srow")
            nc.vector.tensor_copy(out=srow, in_=yu_t[D : D + 1, :])
            nc.sync.dma_start(out=s_dram[h : h + 1, cols], in_=srow)
            # unnormalized y -> bf16 SBUF (normalized in place later)
            nc.vector.tensor_copy(out=Y[pb : pb + D, et, cols], in_=yu_t[0:D, :])

        def attention_head(h, pads, n_pads, last_pair=False):
            et = h // 2
            pb = (h % 2) * 64
            yu = [None, None]
            pts = {}

            def emit_S(kt):
                qlen = L - kt * P
                st = sp.tile([P, L], f32, tag="st", name="st")
                for s0 in range(0, qlen, 512):
                    n = min(512, qlen - s0)
                    nc.tensor.matmul(
                        st[:, s0 : s0 + n],
                        KT[pb : pb + D, et, kt * P : (kt + 1) * P],
                        QT[pb : pb + D, et, kt * P + s0 : kt * P + s0 + n],
                        start=True,
                        stop=True,
                    )
                pt = pt_pool.tile([P, L], bf16, tag="pt", name="pt")
                nc.scalar.activation(
                    out=pt[:, 0:qlen], in_=st[:, 0:qlen], func=AF.Exp,
                    scale=float(SCALE),
                )
                # causal mask on the diagonal tile
                nc.vector.tensor_tensor(
                    out=pt[:, 0:P], in0=pt[:, 0:P], in1=mask01, op=OP.mult
                )
                pts[kt] = pt

            def emit_A(kt):
                for qc in range(2):
                    lo = max(qc * 512, kt * P)
                    hi = (qc + 1) * 512
                    if lo >= hi:
                        continue
                    if kt == 0 and yu[qc] is None:
                        yu[qc] = yp.tile([D + 1, 512], f32, tag="yu", name="yu")
                    last_kt = min(NT - 1, (qc + 1) * 4 - 1)
                    nc.tensor.matmul(
                        yu[qc][:, lo - qc * 512 : hi - qc * 512],
                        V[:, kt, h, :],
                        pts[kt][:, lo - kt * P : hi - kt * P],
                        start=(kt == 0),
                        stop=(kt == last_kt),
                    )
                    if kt == last_kt:
                        head_tail(h, qc, yu[qc], last_pair)

            emit_S(0)
            emit_S(1)
            for kt in range(NT):
                emit_A(kt)
                if kt + 2 < NT:
                    emit_S(kt + 2)
                # batched pads every other tile: fewer stationary-shape
                # switches (each costs ~100-200 ns on the PE) while still
                # covering ScalarE's exp deficit
                if kt % 2 == 1:
                    for _ in range(n_pads):
                        if pads:
                            pads.popleft()()

        def normalize_et(et):
            for half in range(2):
                hh = 2 * et + half
                bsrc = bass.AP(
                    tensor=s_dram.tensor,
                    offset=s_dram[hh : hh + 1, :].offset,
                    ap=[[0, 64], [1, L]],
                )
                nc.gpsimd.dma_start(
                    out=R[half * 64 : (half + 1) * 64, et, :], in_=bsrc
                )
            nc.vector.reciprocal_approx_fast(out=R[:, et, :], in_=R[:, et, :])
            for half in range(2):
                rows = slice(half * 64, (half + 1) * 64)
                nc.vector.tensor_tensor(
                    out=Y[rows, et, :],
                    in0=Y[rows, et, :],
                    in1=R[rows, et, :],
                    op=OP.mult,
                )

        # ---- et loop: attention pair + interleaved next-et projections ----
        for et in range(NT):
            if et == 2:
                # weight prefetches for the output projection, mid-loop
                nc.sync.dma_start(out=wo_r, in_=wo)
                bo_bc = consts.tile([P, E], f32)
                nc.gpsimd.dma_start(
                    out=bo_bc,
                    in_=bass.AP(
                        tensor=bo.tensor, offset=bo.offset, ap=[[0, P], [1, E]]
                    ),
                )
            last_pair = et == NT - 1
            pads = deque(qk_proj_thunks(et + 1)) if not last_pair else deque()
            attention_head(2 * et, pads, 4, last_pair)
            attention_head(2 * et + 1, pads, 4, last_pair)
            while pads:
                pads.popleft()()
            if not last_pair:
                normalize_et(et)

        # ---- out = Y^T.T @ Wo + bo ----
        for lt in range(NT):
            for oc in range(2):
                ps = pp.tile([P, 512], f32, tag="pp", name="ps_out")
                for et in range(NT):
                    nc.tensor.matmul(
                        ps,
                        Y[:, et, lt * P : (lt + 1) * P],
                        wo_r[:, et, oc * 512 : (oc + 1) * 512],
                        start=(et == 0),
                        stop=(et == NT - 1),
                    )
                osb = osb_pool.tile([P, 512], f32)
                nc.vector.tensor_tensor(
                    out=osb, in0=ps, in1=bo_bc[:, oc * 512 : (oc + 1) * 512],
                    op=OP.add,
                )
                nc.sync.dma_start(
                    out=out[lt * P : (lt + 1) * P, oc * 512 : (oc + 1) * 512],
                    in_=osb,
                )


_COMPILED = None


def _get_compiled():
    global _COMPILED
    if _COMPILED is None:
        nc = _build()
        nc.compile()
        _COMPILED = nc
    return _COMPILED


def kernel(x, Wq, bq, Wk, bk, Wv, bv, Wo, bo, _trace=False):
    import ml_dtypes

    bfl = ml_dtypes.bfloat16
    nc = _get_compiled()
    x = np.ascontiguousarray(np.asarray(x, dtype=np.float32).astype(bfl))
    B = x.shape[0]
    assert B == 8 and x.shape[1] == L and x.shape[2] == E

    def _qk_layout(w):
        # [et, p, ct, e']: per-et contiguous [128, 8, 128] stationary blocks
        w = np.asarray(w, np.float32).astype(bfl)
        return np.ascontiguousarray(
            w.reshape(NT, P, NT, P).transpose(2, 1, 0, 3)
        )

    def _pct_layout(w):
        # [p, ct, e]: moving-operand blocks with contraction rows on partitions
        w = np.asarray(w, np.float32).astype(bfl)
        return np.ascontiguousarray(w.reshape(NT, P, E).transpose(1, 0, 2))

    common = {
        "wq": _qk_layout(Wq),
        "wk": _qk_layout(Wk),
        "wv": _pct_layout(Wv),
        "wo": _pct_layout(Wo),
        "bq": np.ascontiguousarray(np.asarray(bq, np.float32)),
        "bk": np.ascontiguousarray(np.asarray(bk, np.float32)),
        "bv": np.ascontiguousarray(np.asarray(bv, np.float32)),
        "bo": np.ascontiguousarray(np.asarray(bo, np.float32)),
    }
    common["mask01"] = np.tril(np.ones((P, P), np.float32)).T.astype(bfl)
    # xt[b]: [p, ct, l] with xt[b][p, ct, l] = x[b, l, ct*128+p]
    xt = np.ascontiguousarray(
        x.transpose(0, 2, 1).reshape(B, NT, P, L).transpose(0, 2, 1, 3)
    )
    in_maps = [dict(common, x=x[i], xt=xt[i]) for i in range(B)]
    res = run_bass_kernel_spmd(nc, in_maps, core_ids=list(range(8)), trace=_trace)
    outp = np.stack([res.results[i]["out"] for i in range(B)])
    if _trace:
        kernel.last_exec_time_ns = res.exec_time_ns
    return outp
